# revision 4
# baseline (speedup 1.0000x reference)
"""Multi-head attention (B=4, S=2048, D=1024, H=16, Dh=64) on 8 TRN2 NeuronCores.

Sharding: core c -> batch b = c//2, head-group g = c%2 (8 heads, output cols
g*512:(g+1)*512).  Each core runs the full attention for its (batch, 8 heads)
slice; host concatenates the per-core [2048, 512] outputs.

Per-core kernel (bf16 compute, f32 accumulation):
  - x [2048,1024] f32 -> cast bf16 -> PE-transpose -> xT [1024(d), 2048(s)]
  - qT/kT = W.T @ xT  (head-dim on partitions; +bias via per-partition add)
  - v natural = xT.T @ Wv (+bias via K=1 ones-matmul), augmented with a
    ones-column per head -> AV matmul also produces softmax denominators
  - per head: scoresT[sk,sq] = kT.T@qT (K=64); exp on ScalarE with the
    1/sqrt(1024) scale folded in (scores are O(1), no max-subtraction needed);
    out_hT/denoms accumulate in PSUM over sk chunks
  - PE-transpose [65,128] slabs -> natural [128,64+recip] -> per-partition
    scalar multiply -> out rows
"""

import numpy as np
from contextlib import ExitStack

import concourse.bass as bass
import concourse.bacc as bacc
import concourse.mybir as mybir
import concourse.tile as tile
from concourse.bass_utils import run_bass_kernel_spmd
from concourse.masks import make_identity

F32 = mybir.dt.float32
BF16 = mybir.dt.bfloat16

B, S, D = 4, 2048, 1024
H, DH = 16, 64
N_CORES = 8
HPC = 8          # heads per core
DPC = HPC * DH   # output cols per core = 512
SCALE = 1.0 / 32.0  # 1/sqrt(D)

_CACHE = {}


def _build_program():
    nc = bacc.Bacc("TRN2", target_bir_lowering=False, debug=False)

    x_ext = nc.dram_tensor("x", [S, D], F32, kind="ExternalInput").ap()
    wq_ext = nc.dram_tensor("wq", [D, DPC], F32, kind="ExternalInput").ap()
    wk_ext = nc.dram_tensor("wk", [D, DPC], F32, kind="ExternalInput").ap()
    wv_ext = nc.dram_tensor("wv", [D, DPC], F32, kind="ExternalInput").ap()
    bq_ext = nc.dram_tensor("bq", [DPC], F32, kind="ExternalInput").ap()
    bk_ext = nc.dram_tensor("bk", [DPC], F32, kind="ExternalInput").ap()
    bv_ext = nc.dram_tensor("bv", [DPC], F32, kind="ExternalInput").ap()
    out_ext = nc.dram_tensor("out", [S, DPC], F32, kind="ExternalOutput").ap()

    KD = D // 128       # 8 contraction chunks over d_in
    NS = S // 128       # 16 sequence chunks
    MB = DPC // 128     # 4 output blocks (head pairs)

    with tile.TileContext(nc) as tc, ExitStack() as ctx:
        singles = ctx.enter_context(tc.tile_pool(name="singles", bufs=1))

        identity = singles.tile([128, 128], BF16, tag="identity")
        make_identity(nc, identity)

        ones_row = singles.tile([1, 128], BF16, tag="ones_row")
        nc.vector.memset(ones_row, 1.0)

        # --- biases ---
        bq_col = []
        bk_col = []
        for m in range(MB):
            t = singles.tile([128, 1], F32, tag=f"bq{m}")
            nc.sync.dma_start(
                out=t, in_=bq_ext[m * 128:(m + 1) * 128].rearrange("(p o) -> p o", o=1)
            )
            bq_col.append(t)
            t = singles.tile([128, 1], F32, tag=f"bk{m}")
            nc.sync.dma_start(
                out=t, in_=bk_ext[m * 128:(m + 1) * 128].rearrange("(p o) -> p o", o=1)
            )
            bk_col.append(t)
        bv_f32 = singles.tile([1, DPC], F32, tag="bv_f32")
        nc.sync.dma_start(out=bv_f32, in_=bv_ext.rearrange("(o n) -> o n", o=1))
        bv_row = singles.tile([1, DPC], BF16, tag="bv_row")
        nc.vector.tensor_copy(bv_row, bv_f32)

        # --- weights: load f32, cast to bf16 ---
        w_bf = {}
        with tc.tile_pool(name="wstage", bufs=3) as wstage:
            for name, ext in (("wq", wq_ext), ("wk", wk_ext), ("wv", wv_ext)):
                tiles = []
                for k in range(KD):
                    wf = wstage.tile([128, DPC], F32, tag="wstage")
                    nc.sync.dma_start(out=wf, in_=ext[k * 128:(k + 1) * 128, :])
                    wb = singles.tile([128, DPC], BF16, tag=f"{name}_bf{k}")
                    nc.vector.tensor_copy(wb, wf)
                    tiles.append(wb)
                w_bf[name] = tiles

        # --- x: load, cast, PE-transpose -> xT (8 blocks of [128, S]) ---
        xT = [singles.tile([128, S], BF16, tag=f"xT{j}", name=f"xT{j}") for j in range(KD)]
        with tc.tile_pool(name="xstage", bufs=3) as xstage, \
             tc.tile_pool(name="xt_psum", bufs=4, space="PSUM") as xt_psum:
            for i in range(NS):
                xf = xstage.tile([128, D], F32, tag="xf")
                nc.sync.dma_start(out=xf, in_=x_ext[i * 128:(i + 1) * 128, :])
                xb = xstage.tile([128, D], BF16, tag="xb")
                nc.vector.tensor_copy(xb, xf)
                for j in range(KD):
                    pst = xt_psum.tile([128, 128], BF16, tag="pst")
                    nc.tensor.transpose(pst, xb[:, j * 128:(j + 1) * 128], identity)
                    nc.vector.tensor_copy(xT[j][:, i * 128:(i + 1) * 128], pst)

        # --- projections ---
        qT = [singles.tile([128, S], BF16, tag=f"qT{m}", name=f"qT{m}") for m in range(MB)]
        kT = [singles.tile([128, S], BF16, tag=f"kT{m}", name=f"kTt{m}") for m in range(MB)]
        # v with ones column: [128, 8 heads, 65]
        vsb = [singles.tile([128, HPC, DH + 1], BF16, tag=f"v{i}", name=f"v{i}") for i in range(NS)]

        with tc.tile_pool(name="proj_psum", bufs=4, space="PSUM") as proj_psum:
            for dst, wname, bcol in ((qT, "wq", bq_col), (kT, "wk", bk_col)):
                for m in range(MB):
                    for n in range(S // 512):
                        ps = proj_psum.tile([128, 512], F32, tag="proj")
                        for k in range(KD):
                            nc.tensor.matmul(
                                ps,
                                lhsT=w_bf[wname][k][:, m * 128:(m + 1) * 128],
                                rhs=xT[k][:, n * 512:(n + 1) * 512],
                                start=(k == 0),
                                stop=(k == KD - 1),
                            )
                        nc.vector.tensor_scalar_add(
                            dst[m][:, n * 512:(n + 1) * 512], ps, bcol[m]
                        )
            for i in range(NS):
                ps = proj_psum.tile([128, 512], F32, tag="proj")
                for k in range(KD):
                    nc.tensor.matmul(
                        ps,
                        lhsT=xT[k][:, i * 128:(i + 1) * 128],
                        rhs=w_bf["wv"][k],
                        start=(k == 0),
                        stop=False,
                    )
                # bias as rank-1 update: ones[1,128].T @ bv[1,512]
                nc.tensor.matmul(
                    ps, lhsT=ones_row, rhs=bv_row, start=False, stop=True
                )
                nc.vector.tensor_copy(
                    vsb[i][:, :, 0:DH], ps.rearrange("p (h d) -> p h d", h=HPC)
                )
                nc.vector.memset(vsb[i][:, :, DH:DH + 1], 1.0)

        # --- per-head attention ---
        out_full = [singles.tile([128, DPC], F32, tag=f"of{i}", name=f"of{i}") for i in range(NS)]

        with tc.tile_pool(name="s_psum", bufs=2, space="PSUM") as s_psum, \
             tc.tile_pool(name="o_psum", bufs=1, space="PSUM") as o_psum, \
             tc.tile_pool(name="t_psum", bufs=2, space="PSUM") as t_psum, \
             tc.tile_pool(name="e_pool", bufs=4) as e_pool, \
             tc.tile_pool(name="attn_sb", bufs=2) as attn_sb, \
             tc.tile_pool(name="ot_sb", bufs=4) as ot_sb:
            for hp in range(MB):
                for r in (0, 64):
                    h = hp * 2 + r // 64  # head index within core
                    o_sb = attn_sb.tile([65, S], BF16, tag="o_sb")
                    for t in range(S // 1024):
                        po = o_psum.tile([128, 1024], F32, tag="po")
                        for c in range(NS):
                            psc = s_psum.tile([128, 1024], F32, tag="psc")
                            for half in range(2):
                                nc.tensor.matmul(
                                    psc[:, half * 512:(half + 1) * 512],
                                    lhsT=kT[hp][r:r + 64, c * 128:(c + 1) * 128],
                                    rhs=qT[hp][
                                        r:r + 64,
                                        t * 1024 + half * 512:t * 1024 + (half + 1) * 512,
                                    ],
                                    start=True,
                                    stop=True,
                                )
                            e = e_pool.tile([128, 1024], BF16, tag="e")
                            nc.scalar.activation(
                                e, psc, mybir.ActivationFunctionType.Exp, scale=SCALE
                            )
                            for half in range(2):
                                nc.tensor.matmul(
                                    po[0:65, half * 512:(half + 1) * 512],
                                    lhsT=vsb[c][:, h, :],
                                    rhs=e[:, half * 512:(half + 1) * 512],
                                    start=(c == 0),
                                    stop=(c == NS - 1),
                                )
                        nc.vector.tensor_copy(
                            o_sb[0:64, t * 1024:(t + 1) * 1024], po[0:64, :]
                        )
                        with nc.allow_low_precision(reason="softmax recip in bf16"):
                            nc.vector.reciprocal(
                                o_sb[64:65, t * 1024:(t + 1) * 1024], po[64:65, :]
                            )
                    for c2 in range(NS):
                        pt = t_psum.tile([128, 65], BF16, tag="pt")
                        nc.tensor.transpose(
                            pt,
                            o_sb[:, c2 * 128:(c2 + 1) * 128],
                            identity[0:65, 0:65],
                        )
                        ot = ot_sb.tile([128, 65], F32, tag="ot")
                        nc.vector.tensor_copy(ot, pt)
                        nc.vector.tensor_scalar_mul(
                            out_full[c2][:, h * DH:(h + 1) * DH],
                            ot[:, 0:DH],
                            ot[:, DH:DH + 1],
                        )

        for i in range(NS):
            nc.sync.dma_start(out=out_ext[i * 128:(i + 1) * 128, :], in_=out_full[i])

    nc.compile()
    return nc


def _get_program():
    if "nc" not in _CACHE:
        _CACHE["nc"] = _build_program()
    return _CACHE["nc"]


def kernel(x, Wq, bq, Wk, bk, Wv, bv, _trace=False):
    x = np.ascontiguousarray(np.asarray(x, dtype=np.float32))
    Wq = np.ascontiguousarray(np.asarray(Wq, dtype=np.float32))
    Wk = np.ascontiguousarray(np.asarray(Wk, dtype=np.float32))
    Wv = np.ascontiguousarray(np.asarray(Wv, dtype=np.float32))
    bq = np.ascontiguousarray(np.asarray(bq, dtype=np.float32))
    bk = np.ascontiguousarray(np.asarray(bk, dtype=np.float32))
    bv = np.ascontiguousarray(np.asarray(bv, dtype=np.float32))

    nc = _get_program()

    in_maps = []
    for c in range(N_CORES):
        b, g = c // 2, c % 2
        cols = slice(g * DPC, (g + 1) * DPC)
        in_maps.append(
            {
                "x": x[b],
                "wq": np.ascontiguousarray(Wq[:, cols]),
                "wk": np.ascontiguousarray(Wk[:, cols]),
                "wv": np.ascontiguousarray(Wv[:, cols]),
                "bq": np.ascontiguousarray(bq[cols]),
                "bk": np.ascontiguousarray(bk[cols]),
                "bv": np.ascontiguousarray(bv[cols]),
            }
        )

    res = run_bass_kernel_spmd(nc, in_maps, core_ids=list(range(N_CORES)), trace=_trace)
    _CACHE["last_results"] = res

    out = np.empty((B, S, D), dtype=np.float32)
    for c in range(N_CORES):
        b, g = c // 2, c % 2
        out[b, :, g * DPC:(g + 1) * DPC] = res.results[c]["out"]
    return out


# revision 5
# speedup vs baseline: 1.1435x; 1.1435x over previous
"""Multi-head attention (B=4, S=2048, D=1024, H=16, Dh=64) on 8 TRN2 NeuronCores.

Sharding: core c -> batch b = c//2, head-group g = c%2 (8 heads, output cols
g*512:(g+1)*512).  Each core runs the full attention for its (batch, 8 heads)
slice; host concatenates the per-core [2048, 512] outputs.

Per-core kernel (bf16 compute, f32 accumulation):
  - x [2048,1024] f32 -> cast bf16 -> PE-transpose -> xT [1024(d), 2048(s)]
  - qT/kT = W.T @ xT  (head-dim on partitions; +bias via per-partition add)
  - v natural = xT.T @ Wv (+bias via K=1 ones-matmul), augmented with a
    ones-column per head -> AV matmul also produces softmax denominators
  - per head: scoresT[sk,sq] = kT.T@qT (K=64); exp on ScalarE with the
    1/sqrt(1024) scale folded in (scores are O(1), no max-subtraction needed);
    out_hT/denoms accumulate in PSUM over sk chunks
  - PE-transpose [65,128] slabs -> natural [128,64+recip] -> per-partition
    scalar multiply -> out rows
"""

import numpy as np
from contextlib import ExitStack

import concourse.bass as bass
import concourse.bacc as bacc
import concourse.mybir as mybir
import concourse.tile as tile
from concourse.bass_utils import run_bass_kernel_spmd
from concourse.masks import make_identity

F32 = mybir.dt.float32
BF16 = mybir.dt.bfloat16

B, S, D = 4, 2048, 1024
H, DH = 16, 64
N_CORES = 8
HPC = 8          # heads per core
DPC = HPC * DH   # output cols per core = 512
SCALE = 1.0 / 32.0  # 1/sqrt(D)

_CACHE = {}


def _build_program():
    nc = bacc.Bacc("TRN2", target_bir_lowering=False, debug=False)

    x_ext = nc.dram_tensor("x", [S, D], F32, kind="ExternalInput").ap()
    wq_ext = nc.dram_tensor("wq", [D, DPC], F32, kind="ExternalInput").ap()
    wk_ext = nc.dram_tensor("wk", [D, DPC], F32, kind="ExternalInput").ap()
    wv_ext = nc.dram_tensor("wv", [D, DPC], F32, kind="ExternalInput").ap()
    bq_ext = nc.dram_tensor("bq", [DPC], F32, kind="ExternalInput").ap()
    bk_ext = nc.dram_tensor("bk", [DPC], F32, kind="ExternalInput").ap()
    bv_ext = nc.dram_tensor("bv", [DPC], F32, kind="ExternalInput").ap()
    out_ext = nc.dram_tensor("out", [S, DPC], F32, kind="ExternalOutput").ap()

    KD = D // 128       # 8 contraction chunks over d_in
    NS = S // 128       # 16 sequence chunks
    MB = DPC // 128     # 4 output blocks (head pairs)

    with tile.TileContext(nc) as tc, ExitStack() as ctx:
        singles = ctx.enter_context(tc.tile_pool(name="singles", bufs=1))

        identity = singles.tile([128, 128], BF16, tag="identity")
        make_identity(nc, identity)

        ones_row = singles.tile([1, 128], BF16, tag="ones_row")
        nc.vector.memset(ones_row, 1.0)

        # --- biases ---
        bq_col = []
        bk_col = []
        for m in range(MB):
            t = singles.tile([128, 1], F32, tag=f"bq{m}")
            nc.sync.dma_start(
                out=t, in_=bq_ext[m * 128:(m + 1) * 128].rearrange("(p o) -> p o", o=1)
            )
            bq_col.append(t)
            t = singles.tile([128, 1], F32, tag=f"bk{m}")
            nc.sync.dma_start(
                out=t, in_=bk_ext[m * 128:(m + 1) * 128].rearrange("(p o) -> p o", o=1)
            )
            bk_col.append(t)
        bv_f32 = singles.tile([1, DPC], F32, tag="bv_f32")
        nc.sync.dma_start(out=bv_f32, in_=bv_ext.rearrange("(o n) -> o n", o=1))
        bv_row = singles.tile([1, DPC], BF16, tag="bv_row")
        nc.vector.tensor_copy(bv_row, bv_f32)

        # --- weights: load f32, cast to bf16 ---
        w_bf = {}
        with tc.tile_pool(name="wstage", bufs=3) as wstage:
            for name, ext in (("wq", wq_ext), ("wk", wk_ext), ("wv", wv_ext)):
                tiles = []
                for k in range(KD):
                    wf = wstage.tile([128, DPC], F32, tag="wstage")
                    nc.sync.dma_start(out=wf, in_=ext[k * 128:(k + 1) * 128, :])
                    wb = singles.tile([128, DPC], BF16, tag=f"{name}_bf{k}")
                    nc.vector.tensor_copy(wb, wf)
                    tiles.append(wb)
                w_bf[name] = tiles

        # --- x: load, cast, PE-transpose -> xT (8 blocks of [128, S]) ---
        xT = [singles.tile([128, S], BF16, tag=f"xT{j}", name=f"xT{j}") for j in range(KD)]
        with tc.tile_pool(name="xstage", bufs=3) as xstage, \
             tc.tile_pool(name="xt_psum", bufs=4, space="PSUM") as xt_psum:
            for i in range(NS):
                xf = xstage.tile([128, D], F32, tag="xf")
                nc.sync.dma_start(out=xf, in_=x_ext[i * 128:(i + 1) * 128, :])
                xb = xstage.tile([128, D], BF16, tag="xb")
                nc.vector.tensor_copy(xb, xf)
                for j in range(KD):
                    pst = xt_psum.tile([128, 128], BF16, tag="pst")
                    nc.tensor.transpose(pst, xb[:, j * 128:(j + 1) * 128], identity)
                    nc.vector.tensor_copy(xT[j][:, i * 128:(i + 1) * 128], pst)

        # --- projections ---
        qT = [singles.tile([128, S], BF16, tag=f"qT{m}", name=f"qT{m}") for m in range(MB)]
        kT = [singles.tile([128, S], BF16, tag=f"kT{m}", name=f"kTt{m}") for m in range(MB)]
        # v with ones column: [128, 8 heads, 65]
        vsb = [singles.tile([128, HPC, DH + 1], BF16, tag=f"v{i}", name=f"v{i}") for i in range(NS)]

        with tc.tile_pool(name="proj_psum", bufs=4, space="PSUM") as proj_psum:
            for dst, wname, bcol in ((qT, "wq", bq_col), (kT, "wk", bk_col)):
                for m in range(MB):
                    for n in range(S // 512):
                        ps = proj_psum.tile([128, 512], F32, tag="proj")
                        for k in range(KD):
                            nc.tensor.matmul(
                                ps,
                                lhsT=w_bf[wname][k][:, m * 128:(m + 1) * 128],
                                rhs=xT[k][:, n * 512:(n + 1) * 512],
                                start=(k == 0),
                                stop=(k == KD - 1),
                            )
                        nc.vector.tensor_scalar_add(
                            dst[m][:, n * 512:(n + 1) * 512], ps, bcol[m]
                        )
            for i in range(NS):
                ps = proj_psum.tile([128, 512], F32, tag="proj")
                for k in range(KD):
                    nc.tensor.matmul(
                        ps,
                        lhsT=xT[k][:, i * 128:(i + 1) * 128],
                        rhs=w_bf["wv"][k],
                        start=(k == 0),
                        stop=False,
                    )
                # bias as rank-1 update: ones[1,128].T @ bv[1,512]
                nc.tensor.matmul(
                    ps, lhsT=ones_row, rhs=bv_row, start=False, stop=True
                )
                nc.vector.tensor_copy(
                    vsb[i][:, :, 0:DH], ps.rearrange("p (h d) -> p h d", h=HPC)
                )
                nc.vector.memset(vsb[i][:, :, DH:DH + 1], 1.0)

        # --- per-head attention ---
        out_full = [singles.tile([128, DPC], F32, tag=f"of{i}", name=f"of{i}") for i in range(NS)]

        with tc.tile_pool(name="s_psum", bufs=2, space="PSUM") as s_psum, \
             tc.tile_pool(name="o_psum", bufs=1, space="PSUM") as o_psum, \
             tc.tile_pool(name="t_psum", bufs=2, space="PSUM") as t_psum, \
             tc.tile_pool(name="e_pool", bufs=4) as e_pool, \
             tc.tile_pool(name="attn_sb", bufs=2) as attn_sb, \
             tc.tile_pool(name="ot_sb", bufs=4) as ot_sb:
            for hp in range(MB):
                for r in (0, 64):
                    h = hp * 2 + r // 64  # head index within core
                    o_sb = attn_sb.tile([65, S], BF16, tag="o_sb")
                    for t in range(S // 1024):
                        po = o_psum.tile([128, 1024], F32, tag="po")
                        for c in range(NS):
                            psc = s_psum.tile([128, 1024], F32, tag="psc")
                            for half in range(2):
                                nc.tensor.matmul(
                                    psc[:, half * 512:(half + 1) * 512],
                                    lhsT=kT[hp][r:r + 64, c * 128:(c + 1) * 128],
                                    rhs=qT[hp][
                                        r:r + 64,
                                        t * 1024 + half * 512:t * 1024 + (half + 1) * 512,
                                    ],
                                    start=True,
                                    stop=True,
                                )
                            e = e_pool.tile([128, 1024], BF16, tag="e")
                            nc.scalar.activation(
                                e, psc, mybir.ActivationFunctionType.Exp, scale=SCALE
                            )
                            for half in range(2):
                                nc.tensor.matmul(
                                    po[0:65, half * 512:(half + 1) * 512],
                                    lhsT=vsb[c][:, h, :],
                                    rhs=e[:, half * 512:(half + 1) * 512],
                                    start=(c == 0),
                                    stop=(c == NS - 1),
                                )
                        # single copy moves out rows AND the raw denominator row;
                        # po frees fast so the next accumulation never stalls PE
                        nc.vector.tensor_copy(
                            o_sb[0:65, t * 1024:(t + 1) * 1024], po[0:65, :]
                        )
                    for c2 in range(NS):
                        pt = t_psum.tile([128, 65], BF16, tag="pt")
                        nc.tensor.transpose(
                            pt,
                            o_sb[:, c2 * 128:(c2 + 1) * 128],
                            identity[0:65, 0:65],
                        )
                        ot = ot_sb.tile([128, 65], BF16, tag="ot")
                        nc.vector.tensor_copy(ot, pt)
                        rc = ot_sb.tile([128, 1], F32, tag="rc")
                        nc.vector.reciprocal(rc, ot[:, DH:DH + 1])
                        nc.vector.tensor_scalar_mul(
                            out_full[c2][:, h * DH:(h + 1) * DH],
                            ot[:, 0:DH],
                            rc,
                        )

        for i in range(NS):
            nc.sync.dma_start(out=out_ext[i * 128:(i + 1) * 128, :], in_=out_full[i])

    nc.compile()
    return nc


def _get_program():
    if "nc" not in _CACHE:
        _CACHE["nc"] = _build_program()
    return _CACHE["nc"]


def kernel(x, Wq, bq, Wk, bk, Wv, bv, _trace=False):
    x = np.ascontiguousarray(np.asarray(x, dtype=np.float32))
    Wq = np.ascontiguousarray(np.asarray(Wq, dtype=np.float32))
    Wk = np.ascontiguousarray(np.asarray(Wk, dtype=np.float32))
    Wv = np.ascontiguousarray(np.asarray(Wv, dtype=np.float32))
    bq = np.ascontiguousarray(np.asarray(bq, dtype=np.float32))
    bk = np.ascontiguousarray(np.asarray(bk, dtype=np.float32))
    bv = np.ascontiguousarray(np.asarray(bv, dtype=np.float32))

    nc = _get_program()

    in_maps = []
    for c in range(N_CORES):
        b, g = c // 2, c % 2
        cols = slice(g * DPC, (g + 1) * DPC)
        in_maps.append(
            {
                "x": x[b],
                "wq": np.ascontiguousarray(Wq[:, cols]),
                "wk": np.ascontiguousarray(Wk[:, cols]),
                "wv": np.ascontiguousarray(Wv[:, cols]),
                "bq": np.ascontiguousarray(bq[cols]),
                "bk": np.ascontiguousarray(bk[cols]),
                "bv": np.ascontiguousarray(bv[cols]),
            }
        )

    res = run_bass_kernel_spmd(nc, in_maps, core_ids=list(range(N_CORES)), trace=_trace)
    _CACHE["last_results"] = res

    out = np.empty((B, S, D), dtype=np.float32)
    for c in range(N_CORES):
        b, g = c // 2, c % 2
        out[b, :, g * DPC:(g + 1) * DPC] = res.results[c]["out"]
    return out
